# revision 21
# baseline (speedup 1.0000x reference)
"""Trainium2 Bass kernel for nn_CCG_46273977647541.

Reference pipeline per batch (B=8 -> one NeuronCore each, no cross-core
communication): LayerNorm -> NxN cosine similarity -> density row-sum ->
argmax row as cluster center -> 256->64 projection + relu.

The NxN similarity is never materialized.  With ln_w==1, ln_b==0 (the
spec's deterministic fills) the density factorizes exactly; further, the
mean-shift of the weighted row-sum cancels:

  q_n       = rsqrt(sum_c x_nc^2 - (sum_c x_nc)^2/C)
  A         = sum_m q_m x_m          (uncentered!)
  density_n = q_n * (x_n . A) - q_n * mu_n * sum(A)

v3 design notes (vs v1 57-64us / v2 71us):
  - x is cast f32->bf16 during the HBM load (SWDGE cast DMA).
  - all per-row 256-wide reductions (sum x, sum x^2, dot x.A) run as
    *binary fold trees* on DVE (bf16/fp16, 2x perf mode, ~8 fat
    instructions per pass) -- measured per-tile accumulate ops cost
    ~0.5-0.75us each on this silicon, trees are ~4x cheaper.
  - ScalarE only does batched Square passes (writes fp16) + sqrt + a
    slice of the post-S dot reductions, so the streaming-phase q chain
    has a single cross-engine hop (sqrt) and the Tile scheduler cannot
    serialize phases the way v2's ACT-accum chain did.
  - A accumulates via *paired* matmuls (lhsT = [q_2t | q_2t+1], 512-col
    rhs) into two PSUM banks (even pairs normal, odd pairs with swapped
    lhsT columns) so diagonals land on partition 0 of both banks;
    garbage blocks are never read.
  - numerics (validated in numpy vs f64 reference): worst density err
    0.069 vs min top-2 gap 0.259 (3.8x margin), output rel err ~1e-3.

Infrastructure notes: this walrus build accepts only ONE semaphore wait
per engine instruction and rejects some custom ISA ops; _split_multi_waits
post-processes the BIR JSON to hoist extra waits onto EventSemaphore
carriers and neutralize non-fatal SeqAsserts.
"""

import sys

sys.path.insert(0, "/opt/trn_rl_repo")

from contextlib import ExitStack

import numpy as np

import concourse.bass as bass
import concourse.tile as tile
from concourse import mybir
from concourse.bass_utils import run_bass_kernel_spmd

F32 = mybir.dt.float32
BF16 = mybir.dt.bfloat16
FP16 = mybir.dt.float16
AX = mybir.AxisListType
OP = mybir.AluOpType
ACT = mybir.ActivationFunctionType


def _split_multi_waits(bir_json: bytes) -> bytes:
    """This walrus build accepts at most one semaphore wait per engine
    instruction.  Tile can emit several; hoist all but the last onto
    dedicated EventSemaphore carriers placed immediately before the
    instruction (same engine stream, so semantics are preserved --
    the block order is a topological order of the dep graph)."""
    import json as _json

    bir = _json.loads(bir_json)
    n = 0
    for fn in bir["functions"]:
        for bb in fn["blocks"]:
            new = []
            for inst in bb["instructions"]:
                if inst.get("op_name") == "SeqAssert":
                    inst = {
                        "debug": inst.get("debug", 0),
                        "engine": inst["engine"],
                        "ins": [],
                        "outs": [],
                        "name": inst["name"],
                        "opcode": "EventSemaphore",
                        "sync_info": inst.get("sync_info")
                        or {"on_update": [], "on_wait": []},
                    }
                si = inst.get("sync_info")
                waits = (si or {}).get("on_wait") or []
                if len(waits) > 1:
                    for w in waits[:-1]:
                        n += 1
                        new.append(
                            {
                                "debug": inst.get("debug", 0),
                                "engine": inst["engine"],
                                "ins": [],
                                "outs": [],
                                "name": f"antsplitw-{n}",
                                "opcode": "EventSemaphore",
                                "sync_info": {"on_update": [], "on_wait": [w]},
                            }
                        )
                    si["on_wait"] = [waits[-1]]
                new.append(inst)
            bb["instructions"] = new
    return _json.dumps(bir).encode()


def _install_wait_splitter():
    from concourse import bass_utils as _bu
    from concourse import bass2jax as _b2j

    if getattr(_bu, "_ant_wait_splitter", False):
        return
    _orig = _bu.compile_bir_kernel

    def _patched(bir_json, tmpdir, neff_name="file.neff"):
        return _orig(_split_multi_waits(bir_json), tmpdir, neff_name)

    _bu.compile_bir_kernel = _patched
    _bu._ant_wait_splitter = True
    if getattr(_b2j, "compile_bir_kernel", None) is _orig:
        _b2j.compile_bir_kernel = _patched


_install_wait_splitter()

B, N, C, CR = 8, 4096, 256, 64
P = 128
NT = N // P  # 32 row tiles per core
LN_EPS = 1e-5

# hybrid load: 'cast' chunks stream via SWDGE (casting f32->bf16 inline,
# ~215 GB/s), 'f32' chunks via HWDGE/sync at full HBM rate with a DVE
# cast during the window.  Both queues run concurrently and share HBM.
# DMA chunks (issue order): cast gets ~60% of HBM (its path caps at
# ~215 GB/s), f32/HWDGE takes the rest; both queues run concurrently.
CHUNKS = [
    (0, 8, "cast"), (20, 26, "f32"),
    (8, 14, "cast"), (26, 30, "f32"),
    (14, 20, "cast"), (30, 32, "f32"),
]
# stats groups: (lo, hi, needs_cast, engine) -- engine "gp" reduces on the
# idle GPSIMD Q7s, "dve" uses the fold-tree; late groups stay on DVE so the
# S-critical chain is low-latency.
# (lo, hi, needs_cast, arrival_us): arrival feeds tile_wait_until so the
# scheduler's sim matches the real (slower, concurrent-queue) DMA timing.
SGROUPS = [
    (0, 8, False, 13.5), (8, 14, False, 16.5), (14, 20, False, 19.5),
    (20, 26, True, 12.5), (26, 32, True, 16.0),
]
KACT = 8   # dot tiles reduced on ScalarE (ACT accum); rest via DVE tree
PE_WARM = 10

_CACHE: dict = {}


def _tree_reduce(nc, out_f32, src, ta, tb, sl, w):
    """Pairwise-fold src[:, sl, 0:256] three levels (2x DVE mode), then
    one segmented 1x tensor_reduce [P, w, 32] -> [P, w]."""
    AXX = mybir.AxisListType.X
    nc.vector.tensor_add(ta[:, sl, :], src[:, sl, 0:128], src[:, sl, 128:256])
    nc.vector.tensor_add(tb[:, sl, :], ta[:, sl, 0:64], ta[:, sl, 64:128])
    nc.vector.tensor_add(ta[:, sl, 0:32], tb[:, sl, 0:32], tb[:, sl, 32:64])
    nc.vector.reduce_sum(out=out_f32[:, sl], in_=ta[:, sl, 0:32], axis=AXX)


def _build_nc() -> bass.Bass:
    nc = bass.Bass(enable_asserts=False)
    x_d = nc.declare_dram_parameter("x", [N, C], F32, isOutput=False)
    pw_d = nc.declare_dram_parameter("proj_w", [CR, C], F32, isOutput=False)
    pb_d = nc.declare_dram_parameter("proj_b", [CR], F32, isOutput=False)
    out_d = nc.declare_dram_parameter("out", [CR], F32, isOutput=True)

    covered = sorted(r for lo, hi, _ in CHUNKS for r in range(lo, hi))
    assert covered == list(range(NT))

    with ExitStack() as ctx:
        tc = ctx.enter_context(tile.TileContext(nc))
        small = ctx.enter_context(tc.tile_pool(name="small", bufs=1))
        scrp = ctx.enter_context(tc.tile_pool(name="scr", bufs=4))
        psum = ctx.enter_context(tc.tile_pool(name="ps", bufs=1, space="PSUM"))

        # Row n lives at (partition n//NT, tile n%NT): partition-major so
        # each chunk DMA reads contiguous 1KB*w per partition.
        xb = small.tile([P, NT, C], BF16)
        xbig = small.tile([P, NT, C], F32)    # f32 halves land here first
        SQ = small.tile([P, NT, C], FP16)     # ACT Square output
        PRD = small.tile([P, NT, C], BF16)    # dot products
        TRA = small.tile([P, NT, 128], FP16)  # tree ping
        TRB = small.tile([P, NT, 64], FP16)   # tree pong
        Sb8 = small.tile([P, 8, C], BF16)     # A broadcast, 8x replicated

        SX = small.tile([P, NT], F32)
        SXX = small.tile([P, NT], F32)
        TU = small.tile([P, NT], F32)
        UU = small.tile([P, NT], F32)
        QS = small.tile([P, NT], F32)
        QQ = small.tile([P, NT], F32)
        QQb = small.tile([P, NT], BF16)
        MU = small.tile([P, NT], F32)
        MUb = small.tile([P, NT], BF16)
        SQE = small.tile([P, NT], F32)
        GQ = small.tile([P, NT], F32)
        XS = small.tile([P, NT], F32)
        T1 = small.tile([P, NT], F32)
        DEN = small.tile([P, NT], F32)
        MASK = small.tile([P, NT], F32)
        W1 = small.tile([P, NT], F32)
        scrj = small.tile([P, NT], F32)
        IOTAJ = small.tile([P, NT], F32)

        S_row16 = small.tile([1, C], BF16)
        sumA1 = small.tile([1, 1], F32)
        sumSb = small.tile([P, 1], F32)
        dmax = small.tile([P, 1], F32)
        gm1 = small.tile([1, 1], F32)
        JIDX = small.tile([P, 1], F32)
        j32 = small.tile([1, 1], mybir.dt.int32)
        w1sel = small.tile([P, 1], F32)
        w1sel16 = small.tile([P, 1], BF16)
        cen16 = small.tile([1, C], BF16)
        cenb = small.tile([CR, C], BF16)
        pw_sb = small.tile([CR, C], F32)
        pb_col = small.tile([CR, 1], F32)
        scr2 = small.tile([CR, C], F32)
        o_sb = small.tile([CR, 1], F32)
        o_row = small.tile([1, CR], F32)
        warm = small.tile([1, 1], F32)
        eps_sb = small.tile([P, 1], F32)
        ones_sb = small.tile([1, P], F32)
        ones16 = small.tile([1, P], BF16)
        id_sb = small.tile([P, P], F32)
        ii32 = small.tile([P, P], mybir.dt.int32)
        ji32 = small.tile([P, NT], mybir.dt.int32)
        pi32 = small.tile([P, 1], mybir.dt.int32)
        iif = small.tile([P, P], F32)
        pif = small.tile([P, 1], F32)

        acc_ps = psum.tile([1, C], F32, tag="acc")
        dmy_ps = psum.tile([1, 1], F32, tag="dmy")
        sb_ps = psum.tile([P, C], F32, tag="bc")
        sums_ps = psum.tile([P, 1], F32, tag="dmy")
        tr_ps = psum.tile([1, P], F32, tag="mx")
        gmax_ps = psum.tile([P, 1], F32, tag="mx2")
        jtr_ps = psum.tile([1, P], F32, tag="mx")
        cc_ps = psum.tile([1, C], F32, tag="dmy")
        cc2_ps = psum.tile([1, 1], F32, tag="mx2")
        cen_ps = psum.tile([CR, C], F32, tag="bc")
        o_ps = psum.tile([1, CR], F32, tag="mx")

        # ---- x-load DMAs first so nothing delays them on the queues ----
        xv = x_d[:, :].rearrange("(p j) c -> p j c", p=P)
        for lo, hi, mode in CHUNKS:
            sl = slice(lo, hi)
            if mode == "cast":
                nc.gpsimd.dma_start(out=xb[:, sl, :], in_=xv[:, sl, :])
            else:
                nc.sync.dma_start(out=xbig[:, sl, :], in_=xv[:, sl, :])
        nc.sync.dma_start(out=pw_sb, in_=pw_d[:, :])
        nc.sync.dma_start(out=pb_col, in_=pb_d[:, None])

        # ---- Setup: constants, ACT table warm, PE DVFS warm ----
        nc.vector.memset(warm, 1.0)
        nc.vector.memset(eps_sb, LN_EPS)
        nc.vector.memset(ones_sb, 1.0)
        nc.vector.memset(ones16, 1.0)
        nc.gpsimd.iota(ii32, pattern=[[1, P]], base=0, channel_multiplier=0)
        nc.gpsimd.iota(ji32, pattern=[[1, NT]], base=0, channel_multiplier=0)
        nc.gpsimd.iota(pi32, pattern=[[0, 1]], base=0, channel_multiplier=1)
        nc.vector.tensor_copy(IOTAJ, ji32)
        nc.vector.tensor_copy(iif, ii32)
        nc.vector.tensor_copy(pif, pi32)
        nc.vector.tensor_scalar(
            out=id_sb, in0=iif, scalar1=pif, scalar2=None, op0=OP.is_equal
        )
        nc.scalar.activation(out=warm, in_=warm, func=ACT.Sqrt)
        for _ in range(PE_WARM):
            nc.tensor.matmul(
                dmy_ps[:, :], ones16[0:1, 0:1], ones16[0:1, 0:1],
                start=True, stop=True,
            )

        # ---- Streaming phase, per stats group ----
        mm_done = 0  # matmul issue counter: start/stop must follow PE issue order
        for lo, hi, needs_cast, arrival in SGROUPS:
            ctx_wait = tc.tile_wait_until(arrival / 1000.0)
            ctx_wait.__enter__()
            sl = slice(lo, hi)
            w = hi - lo
            xsrc = xbig if needs_cast else xb
            # ScalarE: squares (fp16), batched 4 tiles per instruction;
            # late groups get priority -- they sit on the S critical chain
            import contextlib as _cl
            prio = tc.high_priority() if hi >= 26 else _cl.nullcontext()
            with prio:
                for j0 in range(lo, hi, 4):
                    j1 = min(j0 + 4, hi)
                    nc.scalar.activation(
                        out=SQ[:, j0:j1, :], in_=xsrc[:, j0:j1, :], func=ACT.Square
                    )
            if needs_cast:
                # bf16 copy for matmuls + the post-S dot pass
                for j0 in range(lo, hi, 4):
                    j1 = min(j0 + 4, hi)
                    nc.vector.tensor_copy(xb[:, j0:j1, :], xbig[:, j0:j1, :])
            _tree_reduce(nc, SX, xb, TRA, TRB, sl, w)
            _tree_reduce(nc, SXX, SQ, TRA, TRB, sl, w)
            # q = rsqrt(SXX - SX^2/C)
            nc.vector.scalar_tensor_tensor(
                out=TU[:, sl], in0=SX[:, sl], scalar=1.0 / C, in1=SX[:, sl],
                op0=OP.mult, op1=OP.mult,
            )
            nc.vector.tensor_sub(UU[:, sl], SXX[:, sl], TU[:, sl])
            with tc.high_priority():
                nc.scalar.activation(out=QS[:, sl], in_=UU[:, sl], func=ACT.Sqrt)
            nc.vector.reciprocal(out=QQ[:, sl], in_=QS[:, sl])
            nc.vector.tensor_copy(QQb[:, sl], QQ[:, sl])
            for j in range(lo, hi):
                st, sp = (mm_done == 0), (mm_done == NT - 1)
                mm_done += 1
                nc.tensor.matmul(
                    acc_ps[:, :], QQb[:, j : j + 1], xb[:, j, :],
                    start=st, stop=sp,
                )
            ctx_wait.__exit__(None, None, None)

        # ---- mu + center weight (batched; needed only late in the tail) ----
        nc.vector.tensor_scalar_mul(MU, SX, 1.0 / C)
        nc.vector.tensor_copy(MUb, MU)
        nc.scalar.activation(
            out=SQE, in_=UU, func=ACT.Sqrt, bias=eps_sb[:, 0:1], scale=1.0 / C
        )
        nc.vector.reciprocal(out=GQ, in_=SQE)

        # ---- S finalize + broadcast ----
        nc.vector.tensor_scalar(
            out=S_row16, in0=acc_ps[0:1, :], scalar1=1.0, scalar2=None,
            op0=OP.mult, op1=OP.add, accum_out=sumA1,
        )
        nc.tensor.matmul(sb_ps[:, :], ones16[0:1, :], S_row16[0:1, :], start=True, stop=True)
        nc.scalar.copy(out=Sb8[:, 0, :], in_=sb_ps[:, :])
        nc.vector.tensor_copy(Sb8[:, 1, :], Sb8[:, 0, :])
        nc.vector.tensor_copy(Sb8[:, 2:4, :], Sb8[:, 0:2, :])
        nc.vector.tensor_copy(Sb8[:, 4:8, :], Sb8[:, 0:4, :])
        nc.tensor.matmul(sums_ps[:, :], ones_sb[0:1, :], sumA1[0:1, :], start=True, stop=True)
        nc.scalar.copy(out=sumSb, in_=sums_ps[:, :])

        # ---- dot pass: XS[n] = x_n . A ----
        # products, 8 tiles per instruction; tiles 0..KACT-1 (first instr)
        # are reduced on ScalarE while DVE trees the rest
        for j0 in range(0, NT, 8):
            nc.vector.tensor_mul(PRD[:, j0 : j0 + 8, :], xb[:, j0 : j0 + 8, :], Sb8)
        for j in range(KACT):
            junk = scrp.tile([P, C], BF16, tag="ja")
            nc.scalar.activation(
                out=junk, in_=PRD[:, j, :], func=ACT.Copy,
                accum_out=XS[:, j : j + 1],
            )
        sl2 = slice(KACT, NT)
        _tree_reduce(nc, XS, PRD, TRA, TRB, sl2, NT - KACT)

        # ---- density + global argmax mask ----
        nc.vector.tensor_scalar(
            out=T1, in0=MU, scalar1=sumSb, scalar2=None, op0=OP.mult
        )
        nc.vector.tensor_sub(T1, XS, T1)
        nc.vector.tensor_mul(DEN, T1, QQ)
        nc.vector.reduce_max(out=dmax, in_=DEN, axis=AX.X)
        nc.tensor.transpose(tr_ps[:, :], dmax[:, 0:1], id_sb[:, :])
        nc.vector.reduce_max(out=gm1, in_=tr_ps[0:1, :], axis=AX.X)
        nc.tensor.matmul(
            gmax_ps[:, :], ones_sb[0:1, :], gm1[0:1, 0:1], start=True, stop=True
        )
        nc.vector.tensor_scalar(
            out=MASK, in0=DEN, scalar1=gmax_ps[:, 0:1], scalar2=None, op0=OP.is_equal
        )

        # ---- center = sum_p w1[p] * (x[p, j*, :] - mu[p, j*]) ----
        nc.vector.tensor_mul(W1, MASK, GQ)
        nc.vector.reduce_sum(out=w1sel, in_=W1, axis=AX.X)
        nc.vector.tensor_copy(w1sel16, w1sel)
        nc.vector.scalar_tensor_tensor(
            out=scrj, in0=MASK, scalar=1.0, in1=IOTAJ,
            op0=OP.mult, op1=OP.mult, accum_out=JIDX,
        )
        nc.tensor.transpose(jtr_ps[:, :], JIDX[:, 0:1], id_sb[:, :])
        with nc.allow_low_precision(reason="exact small-int index sum"):
            nc.vector.reduce_sum(out=j32, in_=jtr_ps[0:1, :], axis=AX.X)
        jv = nc.tensor.value_load(j32[0:1, 0:1])
        nc.tensor.matmul(
            cc_ps[:, :], w1sel16[:, 0:1], xb[:, bass.ds(jv, 1), :],
            start=True, stop=True,
        )
        nc.tensor.matmul(
            cc2_ps[:, :], w1sel16[:, 0:1], MUb[:, bass.ds(jv, 1)],
            start=True, stop=True,
        )
        nc.vector.tensor_scalar(
            out=cen16, in0=cc_ps[0:1, :], scalar1=cc2_ps[0:1, 0:1], scalar2=None,
            op0=OP.subtract,
        )

        # ---- out = relu(proj_w @ center + proj_b) ----
        nc.tensor.matmul(cen_ps[:, :], ones16[0:1, 0:CR], cen16[0:1, :], start=True, stop=True)
        nc.scalar.copy(out=cenb, in_=cen_ps[:, :])
        nc.vector.scalar_tensor_tensor(
            out=scr2, in0=pw_sb, scalar=1.0, in1=cenb,
            op0=OP.mult, op1=OP.mult, accum_out=o_sb,
        )
        # bias + relu fused on the [CR, 1] column, then transpose for a
        # single contiguous 256B output DMA
        nc.vector.tensor_scalar(
            out=o_sb, in0=o_sb, scalar1=pb_col[:, 0:1], scalar2=0.0,
            op0=OP.add, op1=OP.max,
        )
        nc.tensor.transpose(o_ps[:, :], o_sb[:, 0:1], id_sb[0:CR, 0:CR])
        nc.vector.tensor_copy(o_row, o_ps[0:1, :])
        nc.sync.dma_start(out=out_d[None, :], in_=o_row)

    return nc


def _get_nc() -> bass.Bass:
    if "nc" not in _CACHE:
        _CACHE["nc"] = _build_nc()
    return _CACHE["nc"]


def _ensure_ntff_hook():
    """The image's antenv package lacks axon_hooks; shim it so
    run_bass_kernel_spmd(trace=True) can reach the NTFF profiler."""
    import types

    if "antenv.axon_hooks" in sys.modules:
        return
    m = types.ModuleType("antenv.axon_hooks")
    _hook = [None]
    m.set_axon_ntff_profile_hook = lambda h: _hook.__setitem__(0, h)
    m.get_axon_ntff_profile_hook = lambda: _hook[0]
    sys.modules["antenv.axon_hooks"] = m
    try:
        import antenv

        antenv.axon_hooks = m
        from trn_agent_boot.trn_boot import _ntff_profile_via_ctypes

        m.set_axon_ntff_profile_hook(
            _ntff_profile_via_ctypes("/opt/axon/libaxon_pjrt.so")
        )
    except Exception:
        pass


def _run(x, proj_w, proj_b, trace=False):
    if trace:
        _ensure_ntff_hook()
    nc = _get_nc()
    in_maps = [
        {
            "x": np.ascontiguousarray(x[b], dtype=np.float32),
            "proj_w": np.ascontiguousarray(proj_w, dtype=np.float32),
            "proj_b": np.ascontiguousarray(proj_b, dtype=np.float32),
        }
        for b in range(B)
    ]
    res = run_bass_kernel_spmd(nc, in_maps, list(range(B)), trace=trace)
    out = np.stack([res.results[b]["out"].reshape(1, CR) for b in range(B)])
    return out.astype(np.float32), res


def kernel(x, ln_w, ln_b, proj_w, proj_b):
    x = np.asarray(x)
    ln_w = np.asarray(ln_w)
    ln_b = np.asarray(ln_b)
    proj_w = np.asarray(proj_w)
    proj_b = np.asarray(proj_b)
    if not (np.allclose(ln_w, 1.0) and np.allclose(ln_b, 0.0)):
        # General ln_w/ln_b fallback (never hit with the spec's fills).
        return _kernel_numpy(x, ln_w, ln_b, proj_w, proj_b)
    out, _ = _run(x, proj_w, proj_b, trace=False)
    return out


def _kernel_numpy(x, ln_w, ln_b, proj_w, proj_b):
    x = x.astype(np.float32)
    mu = x.mean(-1, keepdims=True)
    var = x.var(-1, keepdims=True)
    xn = (x - mu) / np.sqrt(var + LN_EPS) * ln_w + ln_b
    nrm = np.linalg.norm(xn, axis=-1, keepdims=True)
    out = []
    for b in range(x.shape[0]):
        cos = (xn[b] @ xn[b].T) / (nrm[b] @ nrm[b].T + 1e-8)
        den = cos.sum(-1)
        mask = (den == den.max()).astype(np.float32)[:, None]
        center = (xn[b] * mask).sum(0)
        out.append(np.maximum(proj_w @ center + proj_b, 0.0))
    return np.stack(out)[:, None, :].astype(np.float32)
